# revision 24
# baseline (speedup 1.0000x reference)
"""GQA attention + RoPE, tensor-parallel across 8 NeuronCores (Bass/Tile).

Model: x(1,2048,2048) -> Q=xWq (32 heads x 64), K/V=xWk/xWv (8 kv heads),
RoPE on q/k, causal softmax attention (GQA: 4 q heads per kv head), out-proj.

Sharding: head-parallel. Core i gets q heads 4i..4i+3 (Wq cols), kv head i
(Wk/Wv cols), Wo rows 256i..256i+256. Each core computes a partial (2048,2048)
output; host sums the 8 partials (the "all-reduce").

v2 (vs 435us baseline): bf16 everywhere on PE inputs (halves DMA + DVE),
RoPE half-swap via DVE stream_shuffle (host permutes head dims so the rope
pair (x1_i,x2_i) sits 16 partitions apart inside a 32-partition quadrant -
legal for scores since q and k share the permutation), causal-trimmed ctx
matmuls, reciprocal_approx_fast for the softmax denominator, and proj/
out-proj matmuls interleaved into the attention j-loop so the PE never sees
a >3.4us gap (HAM stays at K=8/8 instead of oscillating).
"""

import numpy as np
from contextlib import ExitStack

import concourse.bass as bass
from concourse import bacc
import concourse.tile as tile
from concourse import mybir
from concourse.bass_utils import run_bass_kernel_spmd

F32 = mybir.dt.float32
BF16 = mybir.dt.bfloat16
AF = mybir.ActivationFunctionType

S = 2048          # sequence length
D = 2048          # model dim
HD = 64           # head dim
NCORES = 8
QH = 4            # q heads per core
QC = QH * HD      # 256 q columns per core
SC = 512          # seq chunk width
NSC = S // SC     # 4 chunks
KB = D // 128     # 16 feature blocks
SCALE = 1.0 / 8.0  # 1/sqrt(64)
SHUF = list(range(16, 32)) + list(range(16))  # rope pair swap, per quadrant

_NC = None


def _build():
    nc = bacc.Bacc(None)
    xT = nc.declare_dram_parameter("xT", [128, KB, S], BF16, isOutput=False)
    wq = nc.declare_dram_parameter("wq", [128, KB, QC], BF16, isOutput=False)
    wkv = nc.declare_dram_parameter("wkv", [128, KB, 128], BF16, isOutput=False)
    wo = nc.declare_dram_parameter("wo", [128, 2, D], BF16, isOutput=False)
    ctab = nc.declare_dram_parameter("ctab", [128, S], BF16, isOutput=False)
    stab = nc.declare_dram_parameter("stab", [128, S], BF16, isOutput=False)
    trimask = nc.declare_dram_parameter("trimask", [128, 128], BF16,
                                        isOutput=False)
    eye = nc.declare_dram_parameter("eye", [64, 64], BF16, isOutput=False)
    out = nc.declare_dram_parameter("out", [S, D], BF16, isOutput=True)

    with tile.TileContext(nc) as tc, ExitStack() as ctx:
        sb = ctx.enter_context(tc.tile_pool(name="sb", bufs=1))
        wk_ = ctx.enter_context(tc.tile_pool(name="wk", bufs=2))
        pp = ctx.enter_context(tc.tile_pool(name="pp", bufs=1, space="PSUM"))

        # ---- persistent constants ----
        eye_sb = sb.tile([64, 64], BF16)
        nc.sync.dma_start(out=eye_sb, in_=eye[:, :])
        wq_sb = sb.tile([128, KB, QC], BF16)
        x_sb = sb.tile([128, KB, S], BF16)
        # x loaded in column halves: proj(0)/proj(1) only need cols 0-1023,
        # so they start ~2x earlier than with full-row loads
        for kb in range(KB):
            nc.sync.dma_start(out=x_sb[:, kb, 0:2 * SC], in_=xT[:, kb, 0:2 * SC])
            nc.sync.dma_start(out=wq_sb[:, kb, :], in_=wq[:, kb, :])
        wkv_sb = sb.tile([128, KB, 128], BF16)
        nc.sync.dma_start(out=wkv_sb, in_=wkv[:, :, :])
        ctab_sb = sb.tile([128, S], BF16)
        nc.sync.dma_start(out=ctab_sb, in_=ctab[:, :])
        stab_sb = sb.tile([128, S], BF16)
        nc.sync.dma_start(out=stab_sb, in_=stab[:, :])
        for kb in range(KB):
            nc.sync.dma_start(out=x_sb[:, kb, 2 * SC:S],
                              in_=xT[:, kb, 2 * SC:S])
        tri_sb = sb.tile([128, 128], BF16)
        nc.sync.dma_start(out=tri_sb, in_=trimask[:, :])
        wo_sb = sb.tile([128, 2, D], BF16)
        nc.sync.dma_start(out=wo_sb, in_=wo[:, :, :])

        # PE warmup spin: keep the PE busy from t~1us so the HAM clock gate
        # opens (K=8/8) before the projection matmuls start, and bridge the
        # DMA-gated prologue.
        warm = pp.tile([64, 64], F32, name="warm", tag="fill", bufs=1)
        for _ in range(64):
            nc.tensor.matmul(warm, lhsT=eye_sb, rhs=eye_sb,
                             start=True, stop=True)

        _burst_id = [0]

        def emit_warm_burst(n):
            """Dependency-free N=512 matmuls emitted into a known PE bubble
            (chunk boundary / normalize barrier). They convert idle time into
            HAM activity so the clock gate re-opens to K=8/8 instead of the
            whole next phase running at 1.2 GHz."""
            _burst_id[0] += 1
            wb = pp.tile([64, SC], F32, name=f"wb_{_burst_id[0]}", tag="fill",
                         bufs=1)
            for _ in range(n):
                nc.tensor.matmul(wb, lhsT=eye_sb, rhs=ctab_sb[0:64, 0:SC],
                                 start=True, stop=True)

        # ---- persistent activations ----
        qt0 = sb.tile([128, S], BF16)   # q^T heads 0,1 (roped)
        qt1 = sb.tile([128, S], BF16)   # q^T heads 2,3
        qts = [qt0, qt1]
        kt_sb = sb.tile([128, S], BF16)  # rows 0-63 k^T roped; 64-127 dup
        v_sb = sb.tile([128, KB, HD + 2], BF16)  # V natural + [ones, 0] cols
        ct0 = sb.tile([128, S], BF16)   # normalized ctx^T heads 0,1
        ct1 = sb.tile([128, S], BF16)
        cts = [ct0, ct1]
        nc.vector.memset(v_sb[:, :, HD:HD + 1], 1.0)
        nc.vector.memset(v_sb[:, :, HD + 1:HD + 2], 0.0)

        def emit_proj_q(c, u):
            """Q projection + rope for u-tile (2 heads) of chunk c."""
            cs = slice(c * SC, (c + 1) * SC)
            pq = pp.tile([128, SC], F32, name=f"pq_{c}_{u}", tag="fill", bufs=1)
            for kb in range(KB):
                nc.tensor.matmul(
                    pq,
                    lhsT=wq_sb[:, kb, u * 128:(u + 1) * 128],
                    rhs=x_sb[:, kb, cs],
                    start=(kb == 0), stop=(kb == KB - 1),
                )
            qraw = wk_.tile([128, SC], BF16, name=f"qraw_{c}_{u}", tag="qraw",
                            bufs=2)
            nc.vector.tensor_copy(qraw, pq)
            qsw = wk_.tile([128, SC], BF16, name=f"qsw_{c}_{u}", tag="qsw",
                           bufs=2)
            nc.vector.stream_shuffle(qsw, qraw, SHUF)
            t1 = wk_.tile([128, SC], BF16, name=f"rt1_{c}_{u}", tag="rt1",
                          bufs=2)
            nc.vector.tensor_mul(t1, qraw, ctab_sb[:, cs])
            t2 = wk_.tile([128, SC], BF16, name=f"rt2_{c}_{u}", tag="rt2",
                          bufs=2)
            nc.vector.tensor_mul(t2, qsw, stab_sb[:, cs])
            nc.vector.tensor_add(qts[u][:, cs], t1, t2)

        def emit_proj_kv(c):
            """K/V projection for chunk c: rope K (+dup), V to natural."""
            cs = slice(c * SC, (c + 1) * SC)
            pkv = pp.tile([128, SC], F32, name=f"pkv_{c}", tag="fill", bufs=1)
            for kb in range(KB):
                nc.tensor.matmul(
                    pkv,
                    lhsT=wkv_sb[:, kb, :],
                    rhs=x_sb[:, kb, cs],
                    start=(kb == 0), stop=(kb == KB - 1),
                )
            kvraw = wk_.tile([128, SC], BF16, name=f"kvraw_{c}", tag="qraw",
                             bufs=2)
            nc.vector.tensor_copy(kvraw, pkv)
            ksw = wk_.tile([64, SC], BF16, name=f"ksw_{c}", tag="ksw", bufs=2)
            nc.vector.stream_shuffle(ksw, kvraw[0:64, :], SHUF)
            k1 = wk_.tile([64, SC], BF16, name=f"kr1_{c}", tag="kr1", bufs=2)
            nc.vector.tensor_mul(k1, kvraw[0:64, :], ctab_sb[0:64, cs])
            k2 = wk_.tile([64, SC], BF16, name=f"kr2_{c}", tag="kr2", bufs=2)
            nc.vector.tensor_mul(k2, ksw, stab_sb[0:64, cs])
            nc.vector.tensor_add(kt_sb[0:64, cs], k1, k2)
            nc.sync.dma_start(out=kt_sb[64:128, cs], in_=kt_sb[0:64, cs])
            # V natural layout: move rows 64-127 down, PE-transpose per block
            vtr = wk_.tile([64, SC], BF16, name=f"vtr_{c}", tag="vtr", bufs=2)
            nc.sync.dma_start(out=vtr, in_=kvraw[64:128, :])
            for r in range(4):
                j = 4 * c + r
                pt = pp.tile([128, HD], BF16, name=f"pt_{c}_{r}", tag="sp",
                             bufs=3)
                nc.tensor.transpose(pt, vtr[:, r * 128:(r + 1) * 128], eye_sb)
                nc.vector.tensor_copy(v_sb[:, j, 0:HD], pt)

        def emit_outproj_half(c, mi, half, ptag="fill"):
            """Half (2 n-tiles) of one 128-query row block of the out proj."""
            m = 4 * c + mi
            mb = slice(m * 128, (m + 1) * 128)
            ob = wk_.tile([128, 2 * SC], BF16, name=f"ob_{c}_{mi}_{half}",
                          tag="ob", bufs=2)
            for ni in range(2):
                n = 2 * half + ni
                nck = slice(n * SC, (n + 1) * SC)
                po = pp.tile([128, SC], F32, name=f"po_{c}_{mi}_{n}",
                             tag=ptag, bufs=1)
                for u in range(2):
                    nc.tensor.matmul(
                        po,
                        lhsT=cts[u][:, mb],
                        rhs=wo_sb[:, u, nck],
                        start=(u == 0), stop=(u == 1),
                    )
                nc.vector.tensor_copy(ob[:, ni * SC:(ni + 1) * SC], po)
            nc.sync.dma_start(out=out[mb, half * 2 * SC:(half + 1) * 2 * SC],
                              in_=ob)

        def emit_attn(c, fillers):
            """Attention for chunk c; pops filler emitters to keep PE busy.
            A couple of fillers are held back to cover the normalize chain's
            latency at the end of the j-loop."""
            tail = [fillers.pop() for _ in range(min(2, len(fillers)))]
            if c > 0:
                emit_warm_burst(10)
            njt = 4 * c + 4
            heads = [(u, idx) for u in (0, 1) for idx in (0, 1)]
            cps = {}
            for u, idx in heads:
                cps[(u, idx)] = pp.tile([HD + 2, SC], F32,
                                        name=f"cp_{c}_{u}_{idx}",
                                        tag=f"ctx{2 * u + idx}", bufs=1)
            es_for = {}

            def emit_scores_u(j, u):
                diag = j >= 4 * c
                r = j - 4 * c
                jb = slice(j * 128, (j + 1) * 128)
                lo = 128 * r if diag else 0
                nsl = slice(lo, SC)
                csl = slice(c * SC + lo, (c + 1) * SC)
                sps = []
                for idx in (0, 1):
                    sp = pp.tile([128, SC], F32,
                                 name=f"sp_{c}_{u}_{j}_{idx}",
                                 tag="sp", bufs=3)
                    nc.tensor.matmul(
                        sp[:, nsl],
                        lhsT=kt_sb[idx * 64:idx * 64 + 64, jb],
                        rhs=qts[u][idx * 64:idx * 64 + 64, csl],
                        start=True, stop=True,
                        tile_position=(idx * 64, 0),
                    )
                    sps.append(sp)
                for idx in (0, 1):
                    e = wk_.tile([128, SC], BF16,
                                 name=f"e_{c}_{u}_{j}_{idx}",
                                 tag="es", bufs=8)
                    nc.scalar.activation(e[:, nsl], sps[idx][:, nsl],
                                         AF.Exp, scale=SCALE)
                    if diag:
                        dsl = slice(lo, lo + 128)
                        nc.vector.tensor_mul(e[:, dsl], e[:, dsl], tri_sb)
                    es_for[(u, idx, j)] = (e, nsl)

            def emit_ctx_u(j, u):
                for idx in (0, 1):
                    e, nsl = es_for.pop((u, idx, j))
                    nc.tensor.matmul(
                        cps[(u, idx)][:, nsl],
                        lhsT=v_sb[:, j, :],
                        rhs=e[:, nsl],
                        start=(j == 0), stop=(j == njt - 1),
                    )

            # u-interleaved software pipeline: between a u-pair's score MMs
            # and the next u-pair's (which waits on an exp via the sp-bank
            # rotation) the PE always has ready ctx/filler work queued, so
            # the in-order engine queue never head-of-line blocks.
            emit_scores_u(0, 0)
            emit_scores_u(0, 1)
            for j in range(njt):
                if j + 1 < njt:
                    emit_scores_u(j + 1, 0)
                emit_ctx_u(j, 0)
                if j + 1 < njt:
                    emit_scores_u(j + 1, 1)
                emit_ctx_u(j, 1)
                if fillers:
                    fillers.pop(0)()
            # normalize: cts = ctx / den via recip-broadcast-multiply
            cs = slice(c * SC, (c + 1) * SC)
            for u, idx in heads:
                cp = cps[(u, idx)]
                # NOTE: gpsimd.partition_broadcast must read partition 0 on
                # real HW (reading a sliced row at partition 64 simulates
                # fine but returns garbage on silicon), so the denominator
                # row is first moved to partition 0 with a small DMA.
                # The copy grabs all 65 rows (same ACT cost — free-size
                # driven) so the psum bank frees before the den chain ends.
                scr = wk_.tile([HD + 1, SC], F32,
                               name=f"scr_{c}_{u}_{idx}", tag="scr", bufs=4)
                nc.scalar.copy(scr, cp[0:HD + 1, :])
                den0 = wk_.tile([1, SC], F32, name=f"den_{c}_{u}_{idx}",
                                tag="den", bufs=4)
                nc.sync.dma_start(out=den0, in_=scr[HD:HD + 1, :])
                rec0 = wk_.tile([1, SC], F32, name=f"rec_{c}_{u}_{idx}",
                                tag="rec", bufs=4)
                nc.vector.reciprocal_approx_fast(out=rec0, in_=den0)
                bcf = wk_.tile([64, SC], F32, name=f"bcf_{c}_{u}_{idx}",
                               tag="bcf", bufs=4)
                nc.gpsimd.partition_broadcast(bcf, rec0[0:1, :])
                rsl = slice(idx * 64, idx * 64 + 64)
                nc.vector.scalar_tensor_tensor(
                    cts[u][rsl, cs], scr[0:HD, :], 1.0, bcf,
                    mybir.AluOpType.mult, mybir.AluOpType.mult,
                )
            for f in tail:
                f()
            while fillers:
                fillers.pop(0)()

        # ---- schedule ----
        # proj(0)+proj(1) upfront: dense PE work that warms the HAM while x
        # streams in; proj(c+2) + outproj(c-1) interleave into attn(c)'s
        # j-loop, weighted toward the later (longer, exp-bound) chunks.
        for cc in (0, 1):
            emit_proj_q(cc, 0)
            emit_proj_q(cc, 1)
            emit_proj_kv(cc)
        # filler supply matched to each chunk's exp-bound deficit:
        # attn(0)<-proj(2), attn(1)<-proj(3), attn(2)<-outproj(0),
        # attn(3)<-outproj(1)+outproj(2) (the longest loop gets the most)
        op_halves = lambda cc: [
            (lambda c2=cc, m=mi, h=half: emit_outproj_half(c2, m, h))
            for mi in range(4) for half in (0, 1)]
        proj_units = lambda cc: [
            (lambda c2=cc: emit_proj_q(c2, 0)),
            (lambda c2=cc: emit_proj_q(c2, 1)),
            (lambda c2=cc: emit_proj_kv(c2))]
        emit_attn(0, proj_units(2))
        emit_attn(1, proj_units(3))
        emit_attn(2, op_halves(0))
        emit_attn(3, op_halves(1) + op_halves(2))
        # final out-proj: warm burst covers the normalize barrier, and the
        # 8 accumulate+evacuate pipelines rotate through the now-free ctx
        # banks instead of serializing on one
        emit_warm_burst(10)
        ptags = ["ctx0", "ctx1", "ctx2", "ctx3", "fill"]
        for k, (mi, half) in enumerate(
                [(m, h) for m in range(4) for h in (0, 1)]):
            emit_outproj_half(NSC - 1, mi, half, ptag=ptags[k % len(ptags)])

    nc.finalize()
    return nc


def _get_nc():
    global _NC
    if _NC is None:
        _NC = _build()
    return _NC


def _rope_perm():
    """Head-local (64) permutation: pair (x1_i, x2_i) -> 16 apart in a
    32-partition quadrant. newpos[old] for old in 0..63."""
    newpos = np.empty(64, dtype=np.int64)
    for i in range(32):
        newpos[i] = (i // 16) * 32 + (i % 16)           # x1_i
        newpos[32 + i] = (i // 16) * 32 + 16 + (i % 16)  # x2_i
    return newpos


def _prep_in_maps(x, Wq, Wk, Wv, Wo, cos, sin):
    import ml_dtypes
    bf = ml_dtypes.bfloat16
    x0 = np.asarray(x, np.float32).reshape(S, D)
    xT = np.ascontiguousarray(
        x0.T.reshape(KB, 128, S).transpose(1, 0, 2)).astype(bf)

    newpos = _rope_perm()
    # permutation as gather: perm_src[new] = old
    perm_src = np.empty(64, dtype=np.int64)
    perm_src[newpos] = np.arange(64)

    # rope tables in the permuted layout (pattern has period 64)
    cosT = np.asarray(cos, np.float32).T  # (32, S)
    sinT = np.asarray(sin, np.float32).T
    ctab64 = np.empty((64, S), np.float32)
    stab64 = np.empty((64, S), np.float32)
    for p in range(64):
        quad, off = p // 32, p % 32
        i = quad * 16 + (off % 16)
        is_x2 = off >= 16
        ctab64[p] = cosT[i]
        stab64[p] = sinT[i] if is_x2 else -sinT[i]
    ctab = np.tile(ctab64, (2, 1)).astype(bf)
    stab = np.tile(stab64, (2, 1)).astype(bf)

    trimask = (np.arange(128)[:, None] <= np.arange(128)[None, :]).astype(bf)
    eye = np.eye(64, dtype=np.float32).astype(bf)

    Wq = np.asarray(Wq, np.float32)
    Wk = np.asarray(Wk, np.float32)
    Wv = np.asarray(Wv, np.float32)
    Wo = np.asarray(Wo, np.float32)
    # apply rope perm within each head's 64 columns
    Wq_p = Wq.reshape(D, 32, 64)[:, :, perm_src].reshape(D, D)
    Wk_p = Wk.reshape(D, 8, 64)[:, :, perm_src].reshape(D, 8 * 64)

    in_maps = []
    for i in range(NCORES):
        wq_i = np.ascontiguousarray(
            Wq_p[:, i * QC:(i + 1) * QC].reshape(KB, 128, QC)
            .transpose(1, 0, 2)).astype(bf)
        wkv_i = np.concatenate(
            [Wk_p[:, i * HD:(i + 1) * HD], Wv[:, i * HD:(i + 1) * HD]],
            axis=1)
        wkv_i = np.ascontiguousarray(
            wkv_i.reshape(KB, 128, 128).transpose(1, 0, 2)).astype(bf)
        wo_i = np.ascontiguousarray(
            Wo[i * QC:(i + 1) * QC, :].reshape(2, 128, D)
            .transpose(1, 0, 2)).astype(bf)
        in_maps.append({
            "xT": xT, "wq": wq_i, "wkv": wkv_i, "wo": wo_i,
            "ctab": ctab, "stab": stab, "trimask": trimask, "eye": eye,
        })
    return in_maps


def run(inputs, **kw):
    nc = _get_nc()
    in_maps = _prep_in_maps(**inputs)
    return run_bass_kernel_spmd(nc, in_maps, list(range(NCORES)), **kw)


def kernel(x, Wq, Wk, Wv, Wo, cos, sin):
    res = run(dict(x=x, Wq=Wq, Wk=Wk, Wv=Wv, Wo=Wo, cos=cos, sin=sin))
    acc = np.zeros((S, D), np.float32)
    for r in res.results:
        acc += r["out"].astype(np.float32)
    return acc.reshape(1, S, D)


# revision 25
# speedup vs baseline: 1.0824x; 1.0824x over previous
"""GQA attention + RoPE, tensor-parallel across 8 NeuronCores (Bass/Tile).

Model: x(1,2048,2048) -> Q=xWq (32 heads x 64), K/V=xWk/xWv (8 kv heads),
RoPE on q/k, causal softmax attention (GQA: 4 q heads per kv head), out-proj.

Sharding: head-parallel. Core i gets q heads 4i..4i+3 (Wq cols), kv head i
(Wk/Wv cols), Wo rows 256i..256i+256. Each core computes a partial (2048,2048)
output; host sums the 8 partials (the "all-reduce").

v2 (vs 435us baseline): bf16 everywhere on PE inputs (halves DMA + DVE),
RoPE half-swap via DVE stream_shuffle (host permutes head dims so the rope
pair (x1_i,x2_i) sits 16 partitions apart inside a 32-partition quadrant -
legal for scores since q and k share the permutation), causal-trimmed ctx
matmuls, reciprocal_approx_fast for the softmax denominator, and proj/
out-proj matmuls interleaved into the attention j-loop so the PE never sees
a >3.4us gap (HAM stays at K=8/8 instead of oscillating).
"""

import numpy as np
from contextlib import ExitStack

import concourse.bass as bass
from concourse import bacc
import concourse.tile as tile
from concourse import mybir
from concourse.bass_utils import run_bass_kernel_spmd

F32 = mybir.dt.float32
BF16 = mybir.dt.bfloat16
AF = mybir.ActivationFunctionType

S = 2048          # sequence length
D = 2048          # model dim
HD = 64           # head dim
NCORES = 8
QH = 4            # q heads per core
QC = QH * HD      # 256 q columns per core
SC = 512          # seq chunk width
NSC = S // SC     # 4 chunks
KB = D // 128     # 16 feature blocks
SCALE = 1.0 / 8.0  # 1/sqrt(64)
SHUF = list(range(16, 32)) + list(range(16))  # rope pair swap, per quadrant

_NC = None


def _build():
    nc = bacc.Bacc(None)
    xT = nc.declare_dram_parameter("xT", [128, KB, S], BF16, isOutput=False)
    wq = nc.declare_dram_parameter("wq", [128, KB, QC], BF16, isOutput=False)
    wkv = nc.declare_dram_parameter("wkv", [128, KB, 128], BF16, isOutput=False)
    wo = nc.declare_dram_parameter("wo", [128, 2, D], BF16, isOutput=False)
    ctab = nc.declare_dram_parameter("ctab", [128, S], BF16, isOutput=False)
    stab = nc.declare_dram_parameter("stab", [128, S], BF16, isOutput=False)
    trimask = nc.declare_dram_parameter("trimask", [128, 128], BF16,
                                        isOutput=False)
    eye = nc.declare_dram_parameter("eye", [64, 64], BF16, isOutput=False)
    out = nc.declare_dram_parameter("out", [S, D], BF16, isOutput=True)

    with tile.TileContext(nc) as tc, ExitStack() as ctx:
        sb = ctx.enter_context(tc.tile_pool(name="sb", bufs=1))
        wk_ = ctx.enter_context(tc.tile_pool(name="wk", bufs=2))
        pp = ctx.enter_context(tc.tile_pool(name="pp", bufs=1, space="PSUM"))

        # ---- persistent constants ----
        eye_sb = sb.tile([64, 64], BF16)
        nc.sync.dma_start(out=eye_sb, in_=eye[:, :])
        wq_sb = sb.tile([128, KB, QC], BF16)
        x_sb = sb.tile([128, KB, S], BF16)
        # x loaded in column halves: proj(0)/proj(1) only need cols 0-1023,
        # so they start ~2x earlier than with full-row loads
        for kb in range(KB):
            nc.sync.dma_start(out=x_sb[:, kb, 0:2 * SC], in_=xT[:, kb, 0:2 * SC])
            nc.sync.dma_start(out=wq_sb[:, kb, :], in_=wq[:, kb, :])
        wkv_sb = sb.tile([128, KB, 128], BF16)
        nc.sync.dma_start(out=wkv_sb, in_=wkv[:, :, :])
        ctab_sb = sb.tile([128, S], BF16)
        nc.sync.dma_start(out=ctab_sb, in_=ctab[:, :])
        stab_sb = sb.tile([128, S], BF16)
        nc.sync.dma_start(out=stab_sb, in_=stab[:, :])
        for kb in range(KB):
            nc.sync.dma_start(out=x_sb[:, kb, 2 * SC:S],
                              in_=xT[:, kb, 2 * SC:S])
        tri_sb = sb.tile([128, 128], BF16)
        nc.sync.dma_start(out=tri_sb, in_=trimask[:, :])
        wo_sb = sb.tile([128, 2, D], BF16)
        nc.sync.dma_start(out=wo_sb, in_=wo[:, :, :])

        # PE warmup spin: keep the PE busy from t~1us so the HAM clock gate
        # opens (K=8/8) before the projection matmuls start, and bridge the
        # DMA-gated prologue.
        warm = pp.tile([64, 64], F32, name="warm", tag="ppq", bufs=1)
        for _ in range(64):
            nc.tensor.matmul(warm, lhsT=eye_sb, rhs=eye_sb,
                             start=True, stop=True)

        _burst_id = [0]

        def emit_warm_burst(n):
            """Dependency-free N=512 matmuls emitted into a known PE bubble
            (chunk boundary / normalize barrier). They convert idle time into
            HAM activity so the clock gate re-opens to K=8/8 instead of the
            whole next phase running at 1.2 GHz."""
            _burst_id[0] += 1
            wb = pp.tile([64, SC], F32, name=f"wb_{_burst_id[0]}", tag="po",
                         bufs=1)
            for _ in range(n):
                nc.tensor.matmul(wb, lhsT=eye_sb, rhs=ctab_sb[0:64, 0:SC],
                                 start=True, stop=True)

        # ---- persistent activations ----
        qt0 = sb.tile([128, S], BF16)   # q^T heads 0,1 (roped)
        qt1 = sb.tile([128, S], BF16)   # q^T heads 2,3
        qts = [qt0, qt1]
        kt_sb = sb.tile([128, S], BF16)  # rows 0-63 k^T roped; 64-127 dup
        v_sb = sb.tile([128, KB, HD + 2], BF16)  # V natural + [ones, 0] cols
        ct0 = sb.tile([128, S], BF16)   # normalized ctx^T heads 0,1
        ct1 = sb.tile([128, S], BF16)
        cts = [ct0, ct1]
        nc.vector.memset(v_sb[:, :, HD:HD + 1], 1.0)
        nc.vector.memset(v_sb[:, :, HD + 1:HD + 2], 0.0)

        def emit_proj_q(c, u):
            """Q projection + rope for u-tile (2 heads) of chunk c."""
            cs = slice(c * SC, (c + 1) * SC)
            pq = pp.tile([128, SC], F32, name=f"pq_{c}_{u}", tag="ppq", bufs=1)
            for kb in range(KB):
                nc.tensor.matmul(
                    pq,
                    lhsT=wq_sb[:, kb, u * 128:(u + 1) * 128],
                    rhs=x_sb[:, kb, cs],
                    start=(kb == 0), stop=(kb == KB - 1),
                )
            qraw = wk_.tile([128, SC], BF16, name=f"qraw_{c}_{u}", tag="qraw",
                            bufs=2)
            nc.vector.tensor_copy(qraw, pq)
            qsw = wk_.tile([128, SC], BF16, name=f"qsw_{c}_{u}", tag="qsw",
                           bufs=2)
            nc.vector.stream_shuffle(qsw, qraw, SHUF)
            t1 = wk_.tile([128, SC], BF16, name=f"rt1_{c}_{u}", tag="rt1",
                          bufs=2)
            nc.vector.tensor_mul(t1, qraw, ctab_sb[:, cs])
            t2 = wk_.tile([128, SC], BF16, name=f"rt2_{c}_{u}", tag="rt2",
                          bufs=2)
            nc.vector.tensor_mul(t2, qsw, stab_sb[:, cs])
            nc.vector.tensor_add(qts[u][:, cs], t1, t2)

        def emit_proj_kv(c):
            """K/V projection for chunk c: rope K (+dup), V to natural."""
            cs = slice(c * SC, (c + 1) * SC)
            pkv = pp.tile([128, SC], F32, name=f"pkv_{c}", tag="ppq", bufs=1)
            for kb in range(KB):
                nc.tensor.matmul(
                    pkv,
                    lhsT=wkv_sb[:, kb, :],
                    rhs=x_sb[:, kb, cs],
                    start=(kb == 0), stop=(kb == KB - 1),
                )
            kvraw = wk_.tile([128, SC], BF16, name=f"kvraw_{c}", tag="qraw",
                             bufs=2)
            nc.vector.tensor_copy(kvraw, pkv)
            ksw = wk_.tile([64, SC], BF16, name=f"ksw_{c}", tag="ksw", bufs=2)
            nc.vector.stream_shuffle(ksw, kvraw[0:64, :], SHUF)
            k1 = wk_.tile([64, SC], BF16, name=f"kr1_{c}", tag="kr1", bufs=2)
            nc.vector.tensor_mul(k1, kvraw[0:64, :], ctab_sb[0:64, cs])
            k2 = wk_.tile([64, SC], BF16, name=f"kr2_{c}", tag="kr2", bufs=2)
            nc.vector.tensor_mul(k2, ksw, stab_sb[0:64, cs])
            nc.vector.tensor_add(kt_sb[0:64, cs], k1, k2)
            nc.sync.dma_start(out=kt_sb[64:128, cs], in_=kt_sb[0:64, cs])
            # V natural layout: move rows 64-127 down, PE-transpose per block
            vtr = wk_.tile([64, SC], BF16, name=f"vtr_{c}", tag="vtr", bufs=2)
            nc.sync.dma_start(out=vtr, in_=kvraw[64:128, :])
            for r in range(4):
                j = 4 * c + r
                pt = pp.tile([128, HD], BF16, name=f"pt_{c}_{r}", tag="sp",
                             bufs=2)
                nc.tensor.transpose(pt, vtr[:, r * 128:(r + 1) * 128], eye_sb)
                nc.vector.tensor_copy(v_sb[:, j, 0:HD], pt)

        def emit_outproj_half(c, mi, half, ptag="po"):
            """Half (2 n-tiles) of one 128-query row block of the out proj."""
            m = 4 * c + mi
            mb = slice(m * 128, (m + 1) * 128)
            ob = wk_.tile([128, 2 * SC], BF16, name=f"ob_{c}_{mi}_{half}",
                          tag="ob", bufs=2)
            for ni in range(2):
                n = 2 * half + ni
                nck = slice(n * SC, (n + 1) * SC)
                po = pp.tile([128, SC], F32, name=f"po_{c}_{mi}_{n}",
                             tag=ptag, bufs=1)
                for u in range(2):
                    nc.tensor.matmul(
                        po,
                        lhsT=cts[u][:, mb],
                        rhs=wo_sb[:, u, nck],
                        start=(u == 0), stop=(u == 1),
                    )
                nc.vector.tensor_copy(ob[:, ni * SC:(ni + 1) * SC], po)
            nc.sync.dma_start(out=out[mb, half * 2 * SC:(half + 1) * 2 * SC],
                              in_=ob)

        def emit_attn(c, fillers):
            """Attention for chunk c; pops filler emitters to keep PE busy.
            A couple of fillers are held back to cover the normalize chain's
            latency at the end of the j-loop."""
            tail = [fillers.pop() for _ in range(min(2, len(fillers)))]
            if c > 0:
                emit_warm_burst(10)
            njt = 4 * c + 4
            heads = [(u, idx) for u in (0, 1) for idx in (0, 1)]
            cps = {}
            for u, idx in heads:
                cps[(u, idx)] = pp.tile([HD + 2, SC], F32,
                                        name=f"cp_{c}_{u}_{idx}",
                                        tag=f"ctx{2 * u + idx}", bufs=1)
            es_for = {}

            def emit_scores_u(j, u):
                diag = j >= 4 * c
                r = j - 4 * c
                jb = slice(j * 128, (j + 1) * 128)
                lo = 128 * r if diag else 0
                nsl = slice(lo, SC)
                csl = slice(c * SC + lo, (c + 1) * SC)
                sps = []
                for idx in (0, 1):
                    sp = pp.tile([128, SC], F32,
                                 name=f"sp_{c}_{u}_{j}_{idx}",
                                 tag="sp", bufs=2)
                    nc.tensor.matmul(
                        sp[:, nsl],
                        lhsT=kt_sb[idx * 64:idx * 64 + 64, jb],
                        rhs=qts[u][idx * 64:idx * 64 + 64, csl],
                        start=True, stop=True,
                        tile_position=(idx * 64, 0),
                    )
                    sps.append(sp)
                for idx in (0, 1):
                    e = wk_.tile([128, SC], BF16,
                                 name=f"e_{c}_{u}_{j}_{idx}",
                                 tag="es", bufs=8)
                    nc.scalar.activation(e[:, nsl], sps[idx][:, nsl],
                                         AF.Exp, scale=SCALE)
                    if diag:
                        dsl = slice(lo, lo + 128)
                        nc.vector.tensor_mul(e[:, dsl], e[:, dsl], tri_sb)
                    es_for[(u, idx, j)] = (e, nsl)

            def emit_ctx_u(j, u):
                for idx in (0, 1):
                    e, nsl = es_for.pop((u, idx, j))
                    nc.tensor.matmul(
                        cps[(u, idx)][:, nsl],
                        lhsT=v_sb[:, j, :],
                        rhs=e[:, nsl],
                        start=(j == 0), stop=(j == njt - 1),
                    )

            # u-interleaved software pipeline: between a u-pair's score MMs
            # and the next u-pair's (which waits on an exp via the sp-bank
            # rotation) the PE always has ready ctx/filler work queued, so
            # the in-order engine queue never head-of-line blocks.
            emit_scores_u(0, 0)
            emit_scores_u(0, 1)
            for j in range(njt):
                if j + 1 < njt:
                    emit_scores_u(j + 1, 0)
                emit_ctx_u(j, 0)
                if j + 1 < njt:
                    emit_scores_u(j + 1, 1)
                emit_ctx_u(j, 1)
                if fillers:
                    fillers.pop(0)()
            # normalize: cts = ctx / den via recip-broadcast-multiply
            cs = slice(c * SC, (c + 1) * SC)
            for u, idx in heads:
                cp = cps[(u, idx)]
                # NOTE: gpsimd.partition_broadcast must read partition 0 on
                # real HW (reading a sliced row at partition 64 simulates
                # fine but returns garbage on silicon), so the denominator
                # row is first moved to partition 0 with a small DMA.
                # The copy grabs all 65 rows (same ACT cost — free-size
                # driven) so the psum bank frees before the den chain ends.
                scr = wk_.tile([HD + 1, SC], F32,
                               name=f"scr_{c}_{u}_{idx}", tag="scr", bufs=4)
                nc.scalar.copy(scr, cp[0:HD + 1, :])
                den0 = wk_.tile([1, SC], F32, name=f"den_{c}_{u}_{idx}",
                                tag="den", bufs=4)
                nc.sync.dma_start(out=den0, in_=scr[HD:HD + 1, :])
                rec0 = wk_.tile([1, SC], F32, name=f"rec_{c}_{u}_{idx}",
                                tag="rec", bufs=4)
                nc.vector.reciprocal_approx_fast(out=rec0, in_=den0)
                bcf = wk_.tile([64, SC], F32, name=f"bcf_{c}_{u}_{idx}",
                               tag="bcf", bufs=4)
                nc.gpsimd.partition_broadcast(bcf, rec0[0:1, :])
                rsl = slice(idx * 64, idx * 64 + 64)
                nc.vector.scalar_tensor_tensor(
                    cts[u][rsl, cs], scr[0:HD, :], 1.0, bcf,
                    mybir.AluOpType.mult, mybir.AluOpType.mult,
                )
            for f in tail:
                f()
            while fillers:
                fillers.pop(0)()

        # ---- schedule ----
        # proj(0)+proj(1) upfront: dense PE work that warms the HAM while x
        # streams in; proj(c+2) + outproj(c-1) interleave into attn(c)'s
        # j-loop, weighted toward the later (longer, exp-bound) chunks.
        for cc in (0, 1):
            emit_proj_q(cc, 0)
            emit_proj_q(cc, 1)
            emit_proj_kv(cc)
        # filler supply matched to each chunk's exp-bound deficit:
        # attn(0)<-proj(2), attn(1)<-proj(3), attn(2)<-outproj(0),
        # attn(3)<-outproj(1)+outproj(2) (the longest loop gets the most)
        op_halves = lambda cc: [
            (lambda c2=cc, m=mi, h=half: emit_outproj_half(c2, m, h))
            for mi in range(4) for half in (0, 1)]
        proj_units = lambda cc: [
            (lambda c2=cc: emit_proj_q(c2, 0)),
            (lambda c2=cc: emit_proj_q(c2, 1)),
            (lambda c2=cc: emit_proj_kv(c2))]
        emit_attn(0, proj_units(2))
        emit_attn(1, proj_units(3))
        emit_attn(2, op_halves(0))
        emit_attn(3, op_halves(1) + op_halves(2))
        # final out-proj: warm burst covers the normalize barrier, and the
        # 8 accumulate+evacuate pipelines rotate through the now-free ctx
        # banks instead of serializing on one
        emit_warm_burst(10)
        ptags = ["ctx0", "ctx1", "ctx2", "ctx3", "po"]
        for k, (mi, half) in enumerate(
                [(m, h) for m in range(4) for h in (0, 1)]):
            emit_outproj_half(NSC - 1, mi, half, ptag=ptags[k % len(ptags)])

    nc.finalize()
    return nc


def _get_nc():
    global _NC
    if _NC is None:
        _NC = _build()
    return _NC


def _rope_perm():
    """Head-local (64) permutation: pair (x1_i, x2_i) -> 16 apart in a
    32-partition quadrant. newpos[old] for old in 0..63."""
    newpos = np.empty(64, dtype=np.int64)
    for i in range(32):
        newpos[i] = (i // 16) * 32 + (i % 16)           # x1_i
        newpos[32 + i] = (i // 16) * 32 + 16 + (i % 16)  # x2_i
    return newpos


def _prep_in_maps(x, Wq, Wk, Wv, Wo, cos, sin):
    import ml_dtypes
    bf = ml_dtypes.bfloat16
    x0 = np.asarray(x, np.float32).reshape(S, D)
    xT = np.ascontiguousarray(
        x0.T.reshape(KB, 128, S).transpose(1, 0, 2)).astype(bf)

    newpos = _rope_perm()
    # permutation as gather: perm_src[new] = old
    perm_src = np.empty(64, dtype=np.int64)
    perm_src[newpos] = np.arange(64)

    # rope tables in the permuted layout (pattern has period 64)
    cosT = np.asarray(cos, np.float32).T  # (32, S)
    sinT = np.asarray(sin, np.float32).T
    ctab64 = np.empty((64, S), np.float32)
    stab64 = np.empty((64, S), np.float32)
    for p in range(64):
        quad, off = p // 32, p % 32
        i = quad * 16 + (off % 16)
        is_x2 = off >= 16
        ctab64[p] = cosT[i]
        stab64[p] = sinT[i] if is_x2 else -sinT[i]
    ctab = np.tile(ctab64, (2, 1)).astype(bf)
    stab = np.tile(stab64, (2, 1)).astype(bf)

    trimask = (np.arange(128)[:, None] <= np.arange(128)[None, :]).astype(bf)
    eye = np.eye(64, dtype=np.float32).astype(bf)

    Wq = np.asarray(Wq, np.float32)
    Wk = np.asarray(Wk, np.float32)
    Wv = np.asarray(Wv, np.float32)
    Wo = np.asarray(Wo, np.float32)
    # apply rope perm within each head's 64 columns
    Wq_p = Wq.reshape(D, 32, 64)[:, :, perm_src].reshape(D, D)
    Wk_p = Wk.reshape(D, 8, 64)[:, :, perm_src].reshape(D, 8 * 64)

    in_maps = []
    for i in range(NCORES):
        wq_i = np.ascontiguousarray(
            Wq_p[:, i * QC:(i + 1) * QC].reshape(KB, 128, QC)
            .transpose(1, 0, 2)).astype(bf)
        wkv_i = np.concatenate(
            [Wk_p[:, i * HD:(i + 1) * HD], Wv[:, i * HD:(i + 1) * HD]],
            axis=1)
        wkv_i = np.ascontiguousarray(
            wkv_i.reshape(KB, 128, 128).transpose(1, 0, 2)).astype(bf)
        wo_i = np.ascontiguousarray(
            Wo[i * QC:(i + 1) * QC, :].reshape(2, 128, D)
            .transpose(1, 0, 2)).astype(bf)
        in_maps.append({
            "xT": xT, "wq": wq_i, "wkv": wkv_i, "wo": wo_i,
            "ctab": ctab, "stab": stab, "trimask": trimask, "eye": eye,
        })
    return in_maps


def run(inputs, **kw):
    nc = _get_nc()
    in_maps = _prep_in_maps(**inputs)
    return run_bass_kernel_spmd(nc, in_maps, list(range(NCORES)), **kw)


def kernel(x, Wq, Wk, Wv, Wo, cos, sin):
    res = run(dict(x=x, Wq=Wq, Wk=Wk, Wv=Wv, Wo=Wo, cos=cos, sin=sin))
    acc = np.zeros((S, D), np.float32)
    for r in res.results:
        acc += r["out"].astype(np.float32)
    return acc.reshape(1, S, D)


# revision 28
# speedup vs baseline: 1.3360x; 1.2342x over previous
"""GQA attention + RoPE, tensor-parallel across 8 NeuronCores (Bass/Tile).

Model: x(1,2048,2048) -> Q=xWq (32 heads x 64), K/V=xWk/xWv (8 kv heads),
RoPE on q/k, causal softmax attention (GQA: 4 q heads per kv head), out-proj.

Sharding: head-parallel. Core i gets q heads 4i..4i+3 (Wq cols), kv head i
(Wk/Wv cols), Wo rows 256i..256i+256. Each core computes a partial (2048,2048)
output; host sums the 8 partials (the "all-reduce").

v2 (vs 435us baseline): bf16 everywhere on PE inputs (halves DMA + DVE),
RoPE half-swap via DVE stream_shuffle (host permutes head dims so the rope
pair (x1_i,x2_i) sits 16 partitions apart inside a 32-partition quadrant -
legal for scores since q and k share the permutation), causal-trimmed ctx
matmuls, reciprocal_approx_fast for the softmax denominator, and proj/
out-proj matmuls interleaved into the attention j-loop so the PE never sees
a >3.4us gap (HAM stays at K=8/8 instead of oscillating).
"""

import numpy as np
from contextlib import ExitStack

import concourse.bass as bass
from concourse import bacc
import concourse.tile as tile
from concourse import mybir
from concourse.bass_utils import run_bass_kernel_spmd

F32 = mybir.dt.float32
BF16 = mybir.dt.bfloat16
AF = mybir.ActivationFunctionType

S = 2048          # sequence length
D = 2048          # model dim
HD = 64           # head dim
NCORES = 8
QH = 4            # q heads per core
QC = QH * HD      # 256 q columns per core
SC = 512          # seq chunk width
NSC = S // SC     # 4 chunks
KB = D // 128     # 16 feature blocks
SCALE = 1.0 / 8.0  # 1/sqrt(64)
SHUF = list(range(16, 32)) + list(range(16))  # rope pair swap, per quadrant

_NC = None


def _build():
    nc = bacc.Bacc(None)
    xT = nc.declare_dram_parameter("xT", [128, KB, S], BF16, isOutput=False)
    wq = nc.declare_dram_parameter("wq", [128, KB, QC], BF16, isOutput=False)
    wkv = nc.declare_dram_parameter("wkv", [128, KB, 128], BF16, isOutput=False)
    wo = nc.declare_dram_parameter("wo", [128, 2, D], BF16, isOutput=False)
    ctab = nc.declare_dram_parameter("ctab", [128, S], BF16, isOutput=False)
    stab = nc.declare_dram_parameter("stab", [128, S], BF16, isOutput=False)
    trimask = nc.declare_dram_parameter("trimask", [128, 128], BF16,
                                        isOutput=False)
    eye = nc.declare_dram_parameter("eye", [64, 64], BF16, isOutput=False)
    out = nc.declare_dram_parameter("out", [S, D], BF16, isOutput=True)

    with tile.TileContext(nc) as tc, ExitStack() as ctx:
        sb = ctx.enter_context(tc.tile_pool(name="sb", bufs=1))
        wk_ = ctx.enter_context(tc.tile_pool(name="wk", bufs=2))
        pp = ctx.enter_context(tc.tile_pool(name="pp", bufs=1, space="PSUM"))

        # ---- persistent constants ----
        eye_sb = sb.tile([64, 64], BF16)
        nc.sync.dma_start(out=eye_sb, in_=eye[:, :])
        wq_sb = sb.tile([128, KB, QC], BF16)
        x_sb = sb.tile([128, KB, S], BF16)
        # x loaded in column halves: proj(0)/proj(1) only need cols 0-1023,
        # so they start ~2x earlier than with full-row loads
        for kb in range(KB):
            nc.sync.dma_start(out=x_sb[:, kb, 0:2 * SC], in_=xT[:, kb, 0:2 * SC])
            nc.sync.dma_start(out=wq_sb[:, kb, :], in_=wq[:, kb, :])
        wkv_sb = sb.tile([128, KB, 128], BF16)
        nc.sync.dma_start(out=wkv_sb, in_=wkv[:, :, :])
        ctab_sb = sb.tile([128, S], BF16)
        nc.sync.dma_start(out=ctab_sb, in_=ctab[:, :])
        stab_sb = sb.tile([128, S], BF16)
        nc.sync.dma_start(out=stab_sb, in_=stab[:, :])
        for kb in range(KB):
            nc.sync.dma_start(out=x_sb[:, kb, 2 * SC:S],
                              in_=xT[:, kb, 2 * SC:S])
        tri_sb = sb.tile([128, 128], BF16)
        nc.sync.dma_start(out=tri_sb, in_=trimask[:, :])
        tri2_sb = sb.tile([128, 2, 128], BF16)
        nc.sync.dma_start(out=tri2_sb[:, 0, :], in_=trimask[:, :])
        nc.sync.dma_start(out=tri2_sb[:, 1, :], in_=trimask[:, :])
        wo_sb = sb.tile([128, 2, D], BF16)
        nc.sync.dma_start(out=wo_sb, in_=wo[:, :, :])

        # PE warmup spin: keep the PE busy from t~1us so the HAM clock gate
        # opens (K=8/8) before the projection matmuls start, and bridge the
        # DMA-gated prologue.
        warm = pp.tile([64, 64], F32, name="warm", tag="ppq", bufs=1)
        for _ in range(64):
            nc.tensor.matmul(warm, lhsT=eye_sb, rhs=eye_sb,
                             start=True, stop=True)

        _burst_id = [0]

        def emit_warm_burst(n):
            """Dependency-free N=512 matmuls emitted into a known PE bubble
            (chunk boundary / normalize barrier). They convert idle time into
            HAM activity so the clock gate re-opens to K=8/8 instead of the
            whole next phase running at 1.2 GHz."""
            _burst_id[0] += 1
            wb = pp.tile([64, SC], F32, name=f"wb_{_burst_id[0]}", tag="po",
                         bufs=1)
            for _ in range(n):
                nc.tensor.matmul(wb, lhsT=eye_sb, rhs=ctab_sb[0:64, 0:SC],
                                 start=True, stop=True)

        # ---- persistent activations ----
        qt0 = sb.tile([128, S], BF16)   # q^T heads 0,1 (roped)
        qt1 = sb.tile([128, S], BF16)   # q^T heads 2,3
        qts = [qt0, qt1]
        kt_sb = sb.tile([128, S], BF16)  # rows 0-63 k^T roped; 64-127 dup
        v_sb = sb.tile([128, KB, HD + 2], BF16)  # V natural + [ones, 0] cols
        ct0 = sb.tile([128, S], BF16)   # normalized ctx^T heads 0,1
        ct1 = sb.tile([128, S], BF16)
        cts = [ct0, ct1]
        nc.vector.memset(v_sb[:, :, HD:HD + 1], 1.0)
        nc.vector.memset(v_sb[:, :, HD + 1:HD + 2], 0.0)

        def emit_proj_q(c, u):
            """Q projection + rope for u-tile (2 heads) of chunk c."""
            cs = slice(c * SC, (c + 1) * SC)
            pq = pp.tile([128, SC], F32, name=f"pq_{c}_{u}", tag="ppq", bufs=1)
            for kb in range(KB):
                nc.tensor.matmul(
                    pq,
                    lhsT=wq_sb[:, kb, u * 128:(u + 1) * 128],
                    rhs=x_sb[:, kb, cs],
                    start=(kb == 0), stop=(kb == KB - 1),
                )
            qraw = wk_.tile([128, SC], BF16, name=f"qraw_{c}_{u}", tag="qraw",
                            bufs=2)
            nc.vector.tensor_copy(qraw, pq)
            qsw = wk_.tile([128, SC], BF16, name=f"qsw_{c}_{u}", tag="qsw",
                           bufs=2)
            nc.vector.stream_shuffle(qsw, qraw, SHUF)
            t1 = wk_.tile([128, SC], BF16, name=f"rt1_{c}_{u}", tag="rt1",
                          bufs=2)
            nc.vector.tensor_mul(t1, qraw, ctab_sb[:, cs])
            t2 = wk_.tile([128, SC], BF16, name=f"rt2_{c}_{u}", tag="rt2",
                          bufs=2)
            nc.vector.tensor_mul(t2, qsw, stab_sb[:, cs])
            nc.vector.tensor_add(qts[u][:, cs], t1, t2)

        def emit_proj_kv(c):
            """K/V projection for chunk c: rope K (+dup), V to natural."""
            cs = slice(c * SC, (c + 1) * SC)
            pkv = pp.tile([128, SC], F32, name=f"pkv_{c}", tag="ppq", bufs=1)
            for kb in range(KB):
                nc.tensor.matmul(
                    pkv,
                    lhsT=wkv_sb[:, kb, :],
                    rhs=x_sb[:, kb, cs],
                    start=(kb == 0), stop=(kb == KB - 1),
                )
            kvraw = wk_.tile([128, SC], BF16, name=f"kvraw_{c}", tag="qraw",
                             bufs=2)
            nc.vector.tensor_copy(kvraw, pkv)
            ksw = wk_.tile([64, SC], BF16, name=f"ksw_{c}", tag="ksw", bufs=2)
            nc.vector.stream_shuffle(ksw, kvraw[0:64, :], SHUF)
            k1 = wk_.tile([64, SC], BF16, name=f"kr1_{c}", tag="kr1", bufs=2)
            nc.vector.tensor_mul(k1, kvraw[0:64, :], ctab_sb[0:64, cs])
            k2 = wk_.tile([64, SC], BF16, name=f"kr2_{c}", tag="kr2", bufs=2)
            nc.vector.tensor_mul(k2, ksw, stab_sb[0:64, cs])
            nc.vector.tensor_add(kt_sb[0:64, cs], k1, k2)
            nc.sync.dma_start(out=kt_sb[64:128, cs], in_=kt_sb[0:64, cs])
            # V natural layout: move rows 64-127 down, PE-transpose per block
            vtr = wk_.tile([64, SC], BF16, name=f"vtr_{c}", tag="vtr", bufs=2)
            nc.sync.dma_start(out=vtr, in_=kvraw[64:128, :])
            for r in range(4):
                j = 4 * c + r
                pt = pp.tile([128, HD], BF16, name=f"pt_{c}_{r}", tag="sp",
                             bufs=2)
                nc.tensor.transpose(pt, vtr[:, r * 128:(r + 1) * 128], eye_sb)
                nc.vector.tensor_copy(v_sb[:, j, 0:HD], pt)

        def emit_outproj_half(c, mi, half, ptag="po"):
            """Half (2 n-tiles) of one 128-query row block of the out proj."""
            m = 4 * c + mi
            mb = slice(m * 128, (m + 1) * 128)
            ob = wk_.tile([128, 2 * SC], BF16, name=f"ob_{c}_{mi}_{half}",
                          tag="ob", bufs=2)
            for ni in range(2):
                n = 2 * half + ni
                nck = slice(n * SC, (n + 1) * SC)
                po = pp.tile([128, SC], F32, name=f"po_{c}_{mi}_{n}",
                             tag=ptag, bufs=1)
                for u in range(2):
                    nc.tensor.matmul(
                        po,
                        lhsT=cts[u][:, mb],
                        rhs=wo_sb[:, u, nck],
                        start=(u == 0), stop=(u == 1),
                    )
                nc.vector.tensor_copy(ob[:, ni * SC:(ni + 1) * SC], po)
            nc.sync.dma_start(out=out[mb, half * 2 * SC:(half + 1) * 2 * SC],
                              in_=ob)

        def emit_attn(c, fillers):
            """Attention for chunk c in two passes (u=0 heads, then u=1).
            Each pass pairs its two heads' scores into one 2-bank psum tile
            so a single exp instruction covers both (halves ACT instruction
            overhead), and the first pass's normalize chains hide behind the
            second pass. Fillers keep the PE dense; a couple are held back
            for the final normalize barrier."""
            tail = [fillers.pop() for _ in range(min(2, len(fillers)))]
            if c > 0:
                emit_warm_burst(10)
            njt = 4 * c + 4
            cs = slice(c * SC, (c + 1) * SC)
            for u in (0, 1):
                cps = {}
                for idx in (0, 1):
                    cps[idx] = pp.tile([HD + 2, SC], F32,
                                       name=f"cp_{c}_{u}_{idx}",
                                       tag=f"ctx{idx}", bufs=1)
                es_for = {}

                def emit_scores_p(j, u=u, es_for=es_for):
                    diag = j >= 4 * c
                    r = j - 4 * c
                    jb = slice(j * 128, (j + 1) * 128)
                    lo = 128 * r if diag else 0
                    nsl = slice(lo, SC)
                    csl = slice(c * SC + lo, (c + 1) * SC)
                    sp = pp.tile([128, 2, SC], F32, name=f"sp_{c}_{u}_{j}",
                                 tag="sp", bufs=2)
                    for idx in (0, 1):
                        nc.tensor.matmul(
                            sp[:, idx, nsl],
                            lhsT=kt_sb[idx * 64:idx * 64 + 64, jb],
                            rhs=qts[u][idx * 64:idx * 64 + 64, csl],
                            start=True, stop=True,
                            tile_position=(idx * 64, 0),
                        )
                    e = wk_.tile([128, 2, SC], BF16, name=f"e_{c}_{u}_{j}",
                                 tag="es", bufs=4)
                    nc.scalar.activation(e[:, :, nsl], sp[:, :, nsl],
                                         AF.Exp, scale=SCALE)
                    if diag:
                        dsl = slice(lo, lo + 128)
                        nc.vector.tensor_mul(e[:, :, dsl], e[:, :, dsl],
                                             tri2_sb)
                    es_for[j] = (e, nsl)

                def emit_ctx_p(j, es_for=es_for, cps=cps):
                    e, nsl = es_for.pop(j)
                    for idx in (0, 1):
                        nc.tensor.matmul(
                            cps[idx][:, nsl],
                            lhsT=v_sb[:, j, :],
                            rhs=e[:, idx, nsl],
                            start=(j == 0), stop=(j == njt - 1),
                        )

                emit_scores_p(0)
                for j in range(njt):
                    if j + 1 < njt:
                        emit_scores_p(j + 1)
                    emit_ctx_p(j)
                    if fillers and j % 2 == u:
                        fillers.pop(0)()
                # normalize this pass's heads: cts = ctx / den
                for idx in (0, 1):
                    cp = cps[idx]
                    # NOTE: gpsimd.partition_broadcast must read partition 0
                    # on real HW (a sliced row at partition 64 simulates fine
                    # but returns garbage on silicon), so the denominator row
                    # is moved to partition 0 with a small DMA. The ACT copy
                    # grabs all 65 rows (same cost - free-size driven) so the
                    # psum bank frees before the den chain completes.
                    scr = wk_.tile([HD + 1, SC], F32,
                                   name=f"scr_{c}_{u}_{idx}", tag="scr",
                                   bufs=4)
                    nc.scalar.copy(scr, cp[0:HD + 1, :])
                    den0 = wk_.tile([1, SC], F32, name=f"den_{c}_{u}_{idx}",
                                    tag="den", bufs=4)
                    nc.sync.dma_start(out=den0, in_=scr[HD:HD + 1, :])
                    rec0 = wk_.tile([1, SC], F32, name=f"rec_{c}_{u}_{idx}",
                                    tag="rec", bufs=4)
                    nc.vector.reciprocal_approx_fast(out=rec0, in_=den0)
                    bcf = wk_.tile([64, SC], F32, name=f"bcf_{c}_{u}_{idx}",
                                   tag="bcf", bufs=4)
                    nc.gpsimd.partition_broadcast(bcf, rec0[0:1, :])
                    rsl = slice(idx * 64, idx * 64 + 64)
                    nc.vector.scalar_tensor_tensor(
                        cts[u][rsl, cs], scr[0:HD, :], 1.0, bcf,
                        mybir.AluOpType.mult, mybir.AluOpType.mult,
                    )
            for f in tail:
                f()
            while fillers:
                fillers.pop(0)()

        # ---- schedule ----
        # proj(0)+proj(1) upfront: dense PE work that warms the HAM while x
        # streams in; proj(c+2) + outproj(c-1) interleave into attn(c)'s
        # j-loop, weighted toward the later (longer, exp-bound) chunks.
        for cc in (0, 1):
            emit_proj_q(cc, 0)
            emit_proj_q(cc, 1)
            emit_proj_kv(cc)
        # filler supply matched to each chunk's exp-bound deficit:
        # attn(0)<-proj(2), attn(1)<-proj(3), attn(2)<-outproj(0),
        # attn(3)<-outproj(1)+outproj(2) (the longest loop gets the most)
        op_halves = lambda cc: [
            (lambda c2=cc, m=mi, h=half: emit_outproj_half(c2, m, h))
            for mi in range(4) for half in (0, 1)]
        proj_units = lambda cc: [
            (lambda c2=cc: emit_proj_q(c2, 0)),
            (lambda c2=cc: emit_proj_q(c2, 1)),
            (lambda c2=cc: emit_proj_kv(c2))]
        emit_attn(0, proj_units(2))
        emit_attn(1, proj_units(3))
        emit_attn(2, op_halves(0))
        emit_attn(3, op_halves(1) + op_halves(2))
        # final out-proj: warm burst covers the normalize barrier, and the
        # 8 accumulate+evacuate pipelines rotate through the now-free ctx
        # banks instead of serializing on one
        emit_warm_burst(10)
        ptags = ["ctx0", "ctx1", "po"]
        for k, (mi, half) in enumerate(
                [(m, h) for m in range(4) for h in (0, 1)]):
            emit_outproj_half(NSC - 1, mi, half, ptag=ptags[k % len(ptags)])

    nc.finalize()
    return nc


def _get_nc():
    global _NC
    if _NC is None:
        _NC = _build()
    return _NC


def _rope_perm():
    """Head-local (64) permutation: pair (x1_i, x2_i) -> 16 apart in a
    32-partition quadrant. newpos[old] for old in 0..63."""
    newpos = np.empty(64, dtype=np.int64)
    for i in range(32):
        newpos[i] = (i // 16) * 32 + (i % 16)           # x1_i
        newpos[32 + i] = (i // 16) * 32 + 16 + (i % 16)  # x2_i
    return newpos


def _prep_in_maps(x, Wq, Wk, Wv, Wo, cos, sin):
    import ml_dtypes
    bf = ml_dtypes.bfloat16
    x0 = np.asarray(x, np.float32).reshape(S, D)
    xT = np.ascontiguousarray(
        x0.T.reshape(KB, 128, S).transpose(1, 0, 2)).astype(bf)

    newpos = _rope_perm()
    # permutation as gather: perm_src[new] = old
    perm_src = np.empty(64, dtype=np.int64)
    perm_src[newpos] = np.arange(64)

    # rope tables in the permuted layout (pattern has period 64)
    cosT = np.asarray(cos, np.float32).T  # (32, S)
    sinT = np.asarray(sin, np.float32).T
    ctab64 = np.empty((64, S), np.float32)
    stab64 = np.empty((64, S), np.float32)
    for p in range(64):
        quad, off = p // 32, p % 32
        i = quad * 16 + (off % 16)
        is_x2 = off >= 16
        ctab64[p] = cosT[i]
        stab64[p] = sinT[i] if is_x2 else -sinT[i]
    ctab = np.tile(ctab64, (2, 1)).astype(bf)
    stab = np.tile(stab64, (2, 1)).astype(bf)

    trimask = (np.arange(128)[:, None] <= np.arange(128)[None, :]).astype(bf)
    eye = np.eye(64, dtype=np.float32).astype(bf)

    Wq = np.asarray(Wq, np.float32)
    Wk = np.asarray(Wk, np.float32)
    Wv = np.asarray(Wv, np.float32)
    Wo = np.asarray(Wo, np.float32)
    # apply rope perm within each head's 64 columns
    Wq_p = Wq.reshape(D, 32, 64)[:, :, perm_src].reshape(D, D)
    Wk_p = Wk.reshape(D, 8, 64)[:, :, perm_src].reshape(D, 8 * 64)

    in_maps = []
    for i in range(NCORES):
        wq_i = np.ascontiguousarray(
            Wq_p[:, i * QC:(i + 1) * QC].reshape(KB, 128, QC)
            .transpose(1, 0, 2)).astype(bf)
        wkv_i = np.concatenate(
            [Wk_p[:, i * HD:(i + 1) * HD], Wv[:, i * HD:(i + 1) * HD]],
            axis=1)
        wkv_i = np.ascontiguousarray(
            wkv_i.reshape(KB, 128, 128).transpose(1, 0, 2)).astype(bf)
        wo_i = np.ascontiguousarray(
            Wo[i * QC:(i + 1) * QC, :].reshape(2, 128, D)
            .transpose(1, 0, 2)).astype(bf)
        in_maps.append({
            "xT": xT, "wq": wq_i, "wkv": wkv_i, "wo": wo_i,
            "ctab": ctab, "stab": stab, "trimask": trimask, "eye": eye,
        })
    return in_maps


def run(inputs, **kw):
    nc = _get_nc()
    in_maps = _prep_in_maps(**inputs)
    return run_bass_kernel_spmd(nc, in_maps, list(range(NCORES)), **kw)


def kernel(x, Wq, Wk, Wv, Wo, cos, sin):
    res = run(dict(x=x, Wq=Wq, Wk=Wk, Wv=Wv, Wo=Wo, cos=cos, sin=sin))
    acc = np.zeros((S, D), np.float32)
    for r in res.results:
        acc += r["out"].astype(np.float32)
    return acc.reshape(1, S, D)
